# revision 3
# baseline (speedup 1.0000x reference)
"""Trainium2 Bass kernel for nn_ConvJac: 32 Jacobi sweeps of a
variable-coefficient 5-point stencil over a batch of 16 images of 512x512.

Strategy
--------
Data-parallel over the batch: 16 images over 8 NeuronCores -> 2 images per
core, no inter-core communication (the stencil never crosses images).

Per-core layout: the 2 images are stacked into a 1024x512 row block; SBUF
partition p holds 8 consecutive rows flattened along the free dim
(8*512 = 4096 f16 values), with 512-element halo columns on both sides
holding the neighbor partitions' boundary rows.  In this layout all four
stencil neighbors are free-dim offset reads (+-1, +-512); the only
cross-partition traffic is a 1-row halo exchange per sweep, done with
SBUF->SBUF DMAs (zero coefficients at image edges absorb every wrap
artifact, so no masking is needed).

Per sweep:
  VectorE   4 f16 tensor_tensor products coeff*u_shift (2x DVE mode; the
            horizontal +-1 shifts are folded into pre-shifted coefficient
            arrays so every DVE operand stays 4-byte aligned)
  TensorE   5 identity matmuls per PSUM bank accumulate the 4 products + b'
            in fp32 (exact adds)
  ScalarE   evacuates PSUM -> u_next (f16) in 2 copies
  DMA       3 halo copies
Coefficients (harmonic-mean face conductivities / diag) are computed on
device from K once at setup, in fp32.
"""

import numpy as np

import concourse.bacc as bacc
import concourse.mybir as mybir
from concourse.tile import TileContext
from concourse.bass_utils import run_bass_kernel_spmd

P = 128          # SBUF partitions
W = 512          # image width
RPP = 8          # rows per partition (1024 rows / 128 partitions)
F = RPP * W      # interior free-dim size (4096)
H0 = W           # halo width (one row)
UW = F + 2 * H0  # u tile width with halos
N_CORES = 8

_prog_cache = {}


def _build(iters: int):
    nc = bacc.Bacc("TRN2", target_bir_lowering=False, name=f"convjac{iters}")
    f32, f16 = mybir.dt.float32, mybir.dt.float16
    mult = mybir.AluOpType.mult

    u_in = nc.dram_tensor("u_in", [P, F], f32, kind="ExternalInput")
    b_in = nc.dram_tensor("b_in", [P, F], f32, kind="ExternalInput")
    k_in = nc.dram_tensor("k_in", [P, F], f32, kind="ExternalInput")
    ident = nc.dram_tensor("ident", [P, P], f16, kind="ExternalInput")
    out = nc.dram_tensor("out", [P, F], f32, kind="ExternalOutput")

    with TileContext(nc) as tc:
        with tc.tile_pool(name="pers", bufs=1) as pers:
            u0 = pers.tile([P, UW], f16, tag="u0")
            u1 = pers.tile([P, UW], f16, tag="u1")
            # Lh[s] = cL[s-1], Rh[s] = cR[s-1]; slots shifted so the DVE
            # products read u at even offsets (alignment for 2x mode).
            Lh = pers.tile([P, F + 2], f16, tag="Lh")
            Rh = pers.tile([P, F + 2], f16, tag="Rh")
            cD = pers.tile([P, F], f16, tag="cD")
            cU = pers.tile([P, F], f16, tag="cU")
            cB = pers.tile([P, F], f16, tag="cB")
            idt = pers.tile([P, P], f16, tag="idt")
            nc.sync.dma_start(out=idt[:], in_=ident[:])

            with tc.tile_pool(name="setup", bufs=1) as sp:
                kst = sp.tile([P, F + H0], f32, tag="kst")
                rt = sp.tile([P, F + 2], f32, tag="rt")
                ut = sp.tile([P, F + H0], f32, tag="ut")
                s1 = sp.tile([P, F], f32, tag="s1")
                s2 = sp.tile([P, F], f32, tag="s2")

                # K with a one-row halo; 1e30 at image bottoms so
                # 1/(lbd + ~0) realizes the Dirichlet face 2K.
                nc.gpsimd.memset(kst[:, F:F + H0], 1e30)
                nc.sync.dma_start(out=kst[:, 0:F], in_=k_in[:])
                nc.sync.dma_start(out=kst[0:63, F:F + H0], in_=k_in[1:64, 0:H0])
                nc.sync.dma_start(out=kst[64:127, F:F + H0], in_=k_in[65:128, 0:H0])
                # lbd = 1/K (in place)
                nc.vector.reciprocal_approx_fast(out=kst[:], in_=kst[:])
                # horizontal half-faces: rt[s] = 1/(lbd[s-1]+lbd[s])
                nc.vector.tensor_add(s1[:], kst[:, 0:F], kst[:, 1:F + 1])
                nc.vector.reciprocal_approx_fast(out=rt[:, 1:F + 1], in_=s1[:])
                nc.vector.memset(rt[:, 0:F + 1:W], 0.0)  # cross-row faces
                nc.vector.memset(rt[:, F + 1:F + 2], 0.0)
                # vertical half-faces: ut[512+x] = 1/(lbd[x]+lbd[x+512])
                nc.vector.tensor_add(s2[:], kst[:, 0:F], kst[:, H0:F + H0])
                nc.vector.reciprocal_approx_fast(out=ut[:, H0:F + H0], in_=s2[:])
                nc.gpsimd.memset(ut[:, 0:H0], 0.0)
                nc.sync.dma_start(out=ut[1:64, 0:H0], in_=ut[0:63, F:F + H0])
                nc.sync.dma_start(out=ut[65:128, 0:H0], in_=ut[64:127, F:F + H0])
                # rd = 1/(half-diagonal)
                nc.vector.tensor_add(s1[:], rt[:, 0:F], rt[:, 1:F + 1])
                nc.vector.tensor_add(s2[:], ut[:, 0:F], ut[:, H0:F + H0])
                nc.vector.tensor_add(s1[:], s1[:], s2[:])
                nc.vector.reciprocal_approx_fast(out=s1[:], in_=s1[:])
                # normalized coefficients (f32 -> f16)
                nc.vector.tensor_mul(Lh[:, 1:F + 1], rt[:, 0:F], s1[:])
                nc.vector.tensor_mul(Rh[:, 1:F + 1], rt[:, 1:F + 1], s1[:])
                nc.vector.tensor_mul(cD[:], ut[:, 0:F], s1[:])
                nc.vector.tensor_mul(cU[:], ut[:, H0:F + H0], s1[:])
                for col in (0, F + 1):
                    nc.gpsimd.memset(Lh[:, col:col + 1], 0.0)
                    nc.gpsimd.memset(Rh[:, col:col + 1], 0.0)
                # b' = b/(2*half-diag)
                nc.sync.dma_start(out=s2[:], in_=b_in[:])
                nc.vector.scalar_tensor_tensor(
                    out=cB[:], in0=s2[:], scalar=0.5, in1=s1[:], op0=mult, op1=mult
                )
                # initial u (f16) + halos
                nc.sync.dma_start(out=s2[:], in_=u_in[:])
                for uu in (u0, u1):
                    nc.gpsimd.memset(uu[:, 0:H0], 0.0)
                    nc.gpsimd.memset(uu[:, F + H0:UW], 0.0)
                nc.vector.tensor_copy(out=u0[:, H0:F + H0], in_=s2[:])
                nc.sync.dma_start(out=u0[1:128, 0:H0], in_=u0[0:127, F:F + H0])
                nc.sync.dma_start(out=u0[0:63, F + H0:UW], in_=u0[1:64, H0:2 * H0])
                nc.sync.dma_start(out=u0[64:127, F + H0:UW], in_=u0[65:128, H0:2 * H0])

            with tc.tile_pool(name="work", bufs=2) as wp, \
                 tc.tile_pool(name="psum", bufs=1, space="PSUM") as pp:
                bufs = [u0, u1]
                for it in range(iters):
                    src = bufs[it % 2]
                    dst = bufs[1 - it % 2]
                    q1 = wp.tile([P, F + 2], f16, tag="q1")
                    q2 = wp.tile([P, F + 2], f16, tag="q2")
                    t3 = wp.tile([P, F], f16, tag="t3")
                    t4 = wp.tile([P, F], f16, tag="t4")
                    # q1[s] = cL[s-1]*u[s-2], q2[s] = cR[s-1]*u[s]
                    nc.vector.tensor_mul(q1[:], Lh[:], src[:, H0 - 2:F + H0])
                    nc.vector.tensor_mul(q2[:], Rh[:], src[:, H0:F + H0 + 2])
                    nc.vector.tensor_mul(t3[:], cD[:], src[:, 0:F])
                    nc.vector.tensor_mul(t4[:], cU[:], src[:, 2 * H0:UW])
                    ps = pp.tile([P, F], f32, tag="ps")
                    for k in range(8):
                        a, e = k * W, k * W + W
                        mm = nc.tensor.matmul
                        mm(ps[:, a:e], lhsT=idt[:], rhs=cB[:, a:e], start=True, stop=False)
                        mm(ps[:, a:e], lhsT=idt[:], rhs=q1[:, a + 1:e + 1], start=False, stop=False)
                        mm(ps[:, a:e], lhsT=idt[:], rhs=q2[:, a + 1:e + 1], start=False, stop=False)
                        mm(ps[:, a:e], lhsT=idt[:], rhs=t3[:, a:e], start=False, stop=False)
                        mm(ps[:, a:e], lhsT=idt[:], rhs=t4[:, a:e], start=False, stop=True)
                    h = F // 2
                    nc.scalar.copy(out=dst[:, H0:H0 + h], in_=ps[:, 0:h])
                    nc.scalar.copy(out=dst[:, H0 + h:H0 + F], in_=ps[:, h:F])
                    nc.sync.dma_start(out=dst[1:128, 0:H0], in_=dst[0:127, F:F + H0])
                    nc.sync.dma_start(out=dst[0:63, F + H0:UW], in_=dst[1:64, H0:2 * H0])
                    nc.sync.dma_start(out=dst[64:127, F + H0:UW], in_=dst[65:128, H0:2 * H0])

                res = bufs[iters % 2]
                ost = wp.tile([P, F], f32, tag="ost", bufs=1)
                nc.vector.tensor_copy(out=ost[:], in_=res[:, H0:F + H0])
                nc.sync.dma_start(out=out[:], in_=ost[:])

    nc.compile()
    return nc


def _get_program(iters: int):
    if iters not in _prog_cache:
        _prog_cache[iters] = _build(iters)
    return _prog_cache[iters]


def _make_in_maps(u, b, K):
    u = np.ascontiguousarray(u, dtype=np.float32)
    b = np.ascontiguousarray(b, dtype=np.float32)
    K = np.ascontiguousarray(K, dtype=np.float32)
    ident = np.eye(P, dtype=np.float16)
    in_maps = []
    for c in range(N_CORES):
        sl = slice(2 * c, 2 * c + 2)
        in_maps.append({
            "u_in": u[sl].reshape(P, F),
            "b_in": b[sl].reshape(P, F),
            "k_in": K[sl].reshape(P, F),
            "ident": ident,
        })
    return in_maps


def kernel(max_iter, u, b, K):
    iters = int(max_iter)
    nc = _get_program(iters)
    in_maps = _make_in_maps(u, b, K)
    res = run_bass_kernel_spmd(nc, in_maps, core_ids=list(range(N_CORES)))
    out = np.concatenate(
        [r["out"].reshape(2, W, W) for r in res.results], axis=0
    ).astype(np.float32)
    return out


# revision 6
# speedup vs baseline: 10.4180x; 10.4180x over previous
"""Trainium2 Bass kernel for nn_ConvJac: 32 Jacobi sweeps of a
variable-coefficient 5-point stencil over a batch of 16 images of 512x512.

Strategy
--------
Data-parallel over the batch: 16 images over 8 NeuronCores -> 2 images per
core, no inter-core communication (the stencil never crosses images).

Per-core layout: the 2 images are stacked into a 1024x512 row block; SBUF
partition p holds 8 consecutive rows flattened along the free dim
(8*512 = 4096 f16 values), with 512-element halo columns on both sides
holding the neighbor partitions' boundary rows.  In this layout all four
stencil neighbors are free-dim offset reads (+-1, +-512); the only
cross-partition traffic is a 1-row halo exchange per sweep, done with
SBUF->SBUF DMAs (zero coefficients at image edges absorb every wrap
artifact, so no masking is needed).

Per sweep:
  VectorE   4 f16 tensor_tensor products coeff*u_shift (2x DVE mode; the
            horizontal +-1 shifts are folded into pre-shifted coefficient
            arrays so every DVE operand stays 4-byte aligned)
  TensorE   5 identity matmuls per PSUM bank accumulate the 4 products + b'
            in fp32 (exact adds)
  ScalarE   evacuates PSUM -> u_next (f16) in 2 copies
  DMA       3 halo copies
Coefficients (harmonic-mean face conductivities / diag) are computed on
device from K once at setup, in fp32.
"""

import numpy as np

import concourse.bacc as bacc
import concourse.mybir as mybir
from concourse.tile import TileContext
from concourse.bass_utils import run_bass_kernel_spmd

P = 128          # SBUF partitions
W = 512          # image width
RPP = 8          # rows per partition (1024 rows / 128 partitions)
F = RPP * W      # interior free-dim size (4096)
H0 = W           # halo width (one row)
UW = F + 2 * H0  # u tile width with halos
N_CORES = 8

_prog_cache = {}


def _build(iters: int):
    nc = bacc.Bacc("TRN2", target_bir_lowering=False, name=f"convjac{iters}")
    f32, f16 = mybir.dt.float32, mybir.dt.float16
    mult = mybir.AluOpType.mult

    u_in = nc.dram_tensor("u_in", [P, F], f32, kind="ExternalInput")
    b_in = nc.dram_tensor("b_in", [P, F], f32, kind="ExternalInput")
    k_in = nc.dram_tensor("k_in", [P, F], f32, kind="ExternalInput")
    ident = nc.dram_tensor("ident", [P, P], f16, kind="ExternalInput")
    out = nc.dram_tensor("out", [P, F], f32, kind="ExternalOutput")

    with TileContext(nc) as tc:
        with tc.tile_pool(name="pers", bufs=1) as pers:
            u0 = pers.tile([P, UW], f16, tag="u0")
            u1 = pers.tile([P, UW], f16, tag="u1")
            # Lh[s] = cL[s-1], Rh[s] = cR[s-1]; slots shifted so the DVE
            # products read u at even offsets (alignment for 2x mode).
            Lh = pers.tile([P, F + 2], f16, tag="Lh")
            Rh = pers.tile([P, F + 2], f16, tag="Rh")
            cD = pers.tile([P, F], f16, tag="cD")
            cU = pers.tile([P, F], f16, tag="cU")
            cB = pers.tile([P, F], f16, tag="cB")
            idt = pers.tile([P, P], f16, tag="idt")
            nc.sync.dma_start(out=idt[:], in_=ident[:])

            with tc.tile_pool(name="setup", bufs=1) as sp:
                kst = sp.tile([P, F + H0], f32, tag="kst")
                rt = sp.tile([P, F + 2], f32, tag="rt")
                ut = sp.tile([P, F + H0], f32, tag="ut")
                s1 = sp.tile([P, F], f32, tag="s1")
                s2 = sp.tile([P, F], f32, tag="s2")

                # K with a one-row halo; 1e30 at image bottoms so
                # 1/(lbd + ~0) realizes the Dirichlet face 2K.
                nc.gpsimd.memset(kst[:, F:F + H0], 1e30)
                nc.sync.dma_start(out=kst[:, 0:F], in_=k_in[:])
                nc.sync.dma_start(out=kst[0:63, F:F + H0], in_=k_in[1:64, 0:H0])
                nc.sync.dma_start(out=kst[64:127, F:F + H0], in_=k_in[65:128, 0:H0])
                # lbd = 1/K (in place)
                nc.vector.reciprocal_approx_fast(out=kst[:], in_=kst[:])
                # horizontal half-faces: rt[s] = 1/(lbd[s-1]+lbd[s])
                nc.vector.tensor_add(s1[:], kst[:, 0:F], kst[:, 1:F + 1])
                nc.vector.reciprocal_approx_fast(out=rt[:, 1:F + 1], in_=s1[:])
                nc.vector.memset(rt[:, 0:F + 1:W], 0.0)  # cross-row faces
                nc.vector.memset(rt[:, F + 1:F + 2], 0.0)
                # vertical half-faces: ut[512+x] = 1/(lbd[x]+lbd[x+512])
                nc.vector.tensor_add(s2[:], kst[:, 0:F], kst[:, H0:F + H0])
                nc.vector.reciprocal_approx_fast(out=ut[:, H0:F + H0], in_=s2[:])
                nc.gpsimd.memset(ut[:, 0:H0], 0.0)
                nc.sync.dma_start(out=ut[1:64, 0:H0], in_=ut[0:63, F:F + H0])
                nc.sync.dma_start(out=ut[65:128, 0:H0], in_=ut[64:127, F:F + H0])
                # rd = 1/(half-diagonal)
                nc.vector.tensor_add(s1[:], rt[:, 0:F], rt[:, 1:F + 1])
                nc.vector.tensor_add(s2[:], ut[:, 0:F], ut[:, H0:F + H0])
                nc.vector.tensor_add(s1[:], s1[:], s2[:])
                nc.vector.reciprocal_approx_fast(out=s1[:], in_=s1[:])
                # normalized coefficients (f32 -> f16)
                nc.vector.tensor_mul(Lh[:, 1:F + 1], rt[:, 0:F], s1[:])
                nc.vector.tensor_mul(Rh[:, 1:F + 1], rt[:, 1:F + 1], s1[:])
                nc.vector.tensor_mul(cD[:], ut[:, 0:F], s1[:])
                nc.vector.tensor_mul(cU[:], ut[:, H0:F + H0], s1[:])
                for col in (0, F + 1):
                    nc.gpsimd.memset(Lh[:, col:col + 1], 0.0)
                    nc.gpsimd.memset(Rh[:, col:col + 1], 0.0)
                # b' = b/(2*half-diag)
                nc.sync.dma_start(out=s2[:], in_=b_in[:])
                nc.vector.scalar_tensor_tensor(
                    out=cB[:], in0=s2[:], scalar=0.5, in1=s1[:], op0=mult, op1=mult
                )
                # initial u (f16) + halos
                nc.sync.dma_start(out=s2[:], in_=u_in[:])
                for uu in (u0, u1):
                    nc.gpsimd.memset(uu[:, 0:H0], 0.0)
                    nc.gpsimd.memset(uu[:, F + H0:UW], 0.0)
                nc.vector.tensor_copy(out=u0[:, H0:F + H0], in_=s2[:])
                nc.sync.dma_start(out=u0[1:128, 0:H0], in_=u0[0:127, F:F + H0])
                nc.sync.dma_start(out=u0[0:63, F + H0:UW], in_=u0[1:64, H0:2 * H0])
                nc.sync.dma_start(out=u0[64:127, F + H0:UW], in_=u0[65:128, H0:2 * H0])

            # Iterations, pipelined in four 2-bank chunks (1024 interior
            # cols each).  The chunk processing order rotates by +1 every
            # sweep: sweep k handles chunks (k, k+1, k+2, k+3) mod 4.  A
            # chunk of sweep k+1 only needs the previous sweep's chunks
            # {c-1, c, c+1}, which under rotation were processed early, so
            # the steady state has no inter-sweep pipeline bubble and DVE
            # runs back-to-back.
            CW = F // 4  # chunk width (1024)
            with tc.tile_pool(name="work", bufs=2) as wp, \
                 tc.tile_pool(name="psum", bufs=1, space="PSUM") as pp:
                bufs = [u0, u1]
                ps = pp.tile([P, F], f32, tag="ps")
                for it in range(iters):
                    src = bufs[it % 2]
                    dst = bufs[1 - it % 2]
                    q1 = wp.tile([P, F + 2], f16, tag="q1")
                    q2 = wp.tile([P, F + 2], f16, tag="q2")
                    t3 = wp.tile([P, F], f16, tag="t3")
                    t4 = wp.tile([P, F], f16, tag="t4")
                    for j in range(4):
                        c = (it + j) % 4          # chunk index this step
                        x0 = c * CW               # interior col base
                        # q-slot write range: [x0, x0+CW+2) for the first
                        # chunk of the sweep (covers its own lead slots) and
                        # for chunk 0 (its predecessor chunk 3 wraps and
                        # never covers slots 0..2); [x0+2, x0+CW+2)
                        # otherwise (the previous chunk of the SAME sweep
                        # already covered slots x0..x0+2).
                        s0 = x0 if (j == 0 or c == 0) else x0 + 2
                        s1 = x0 + CW + 2
                        # q1[s] = cL[s-1]*u[s-2], q2[s] = cR[s-1]*u[s]
                        nc.vector.tensor_mul(
                            q1[:, s0:s1], Lh[:, s0:s1], src[:, H0 - 2 + s0:H0 - 2 + s1])
                        nc.vector.tensor_mul(
                            q2[:, s0:s1], Rh[:, s0:s1], src[:, H0 + s0:H0 + s1])
                        nc.vector.tensor_mul(
                            t3[:, x0:x0 + CW], cD[:, x0:x0 + CW], src[:, x0:x0 + CW])
                        nc.vector.tensor_mul(
                            t4[:, x0:x0 + CW], cU[:, x0:x0 + CW],
                            src[:, 2 * H0 + x0:2 * H0 + x0 + CW])
                        for k in (2 * c, 2 * c + 1):
                            a, e = k * W, k * W + W
                            mm = nc.tensor.matmul
                            mm(ps[:, a:e], lhsT=idt[:], rhs=cB[:, a:e], start=True, stop=False)
                            mm(ps[:, a:e], lhsT=idt[:], rhs=q1[:, a + 1:e + 1], start=False, stop=False)
                            mm(ps[:, a:e], lhsT=idt[:], rhs=q2[:, a + 1:e + 1], start=False, stop=False)
                            mm(ps[:, a:e], lhsT=idt[:], rhs=t3[:, a:e], start=False, stop=False)
                            mm(ps[:, a:e], lhsT=idt[:], rhs=t4[:, a:e], start=False, stop=True)
                        nc.scalar.copy(out=dst[:, H0 + x0:H0 + x0 + CW],
                                       in_=ps[:, x0:x0 + CW])
                        if c == 0:
                            # back halos need dst row 0 (bank 0, just written)
                            nc.sync.dma_start(out=dst[0:63, F + H0:UW],
                                              in_=dst[1:64, H0:2 * H0])
                            nc.sync.dma_start(out=dst[64:127, F + H0:UW],
                                              in_=dst[65:128, H0:2 * H0])
                        if c == 3:
                            # front halo needs dst row 7 (bank 7, just written)
                            nc.sync.dma_start(out=dst[1:128, 0:H0],
                                              in_=dst[0:127, F:F + H0])

                res = bufs[iters % 2]
                ost = wp.tile([P, F], f32, tag="ost", bufs=1)
                nc.vector.tensor_copy(out=ost[:], in_=res[:, H0:F + H0])
                nc.sync.dma_start(out=out[:], in_=ost[:])

    nc.compile()
    return nc


def _get_program(iters: int):
    if iters not in _prog_cache:
        _prog_cache[iters] = _build(iters)
    return _prog_cache[iters]


def _make_in_maps(u, b, K):
    u = np.ascontiguousarray(u, dtype=np.float32)
    b = np.ascontiguousarray(b, dtype=np.float32)
    K = np.ascontiguousarray(K, dtype=np.float32)
    ident = np.eye(P, dtype=np.float16)
    in_maps = []
    for c in range(N_CORES):
        sl = slice(2 * c, 2 * c + 2)
        in_maps.append({
            "u_in": u[sl].reshape(P, F),
            "b_in": b[sl].reshape(P, F),
            "k_in": K[sl].reshape(P, F),
            "ident": ident,
        })
    return in_maps


def kernel(max_iter, u, b, K):
    iters = int(max_iter)
    nc = _get_program(iters)
    in_maps = _make_in_maps(u, b, K)
    res = run_bass_kernel_spmd(nc, in_maps, core_ids=list(range(N_CORES)))
    out = np.concatenate(
        [r["out"].reshape(2, W, W) for r in res.results], axis=0
    ).astype(np.float32)
    return out
